# revision 3
# baseline (speedup 1.0000x reference)
"""Trainium2 Bass kernel for the CinemaScalarImage SIREN/NeRF MLP.

Network (per point, N = 1048576 points, fp32):
  enc = [x, sin(2^k pi x), cos(2^k pi x)]  k=0..9            [N, 63]
  h = sin(30*(. @ W)) chain: enc->128->128->128->128 (W0..W3)
  x4 = h3 @ W4 + b4 [N,16]; density = relu(x4[:, 0]); scal = x4[:, 1:]
  s_in = [scal, enc] [N,78]; h5 = sin(30*(s_in@W5+b5)); h6 = sin(30*(h5@W6+b6))
  s = h6 @ W7 + b7 [N,1].  Returns (s, density).

Mapping to TRN2 (8 cores, pure data parallel over points):
  - feature-major layout: features on SBUF partitions, points on the free dim
  - all sine args computed in "turns" space y = arg/(2pi): weights pre-scaled
    by 30/(2pi) on host, so range reduction is y - round(y) (exact fp32),
    and the ACT Sin evaluates sin(2pi*d + bias) via its free fp32 affine
    (plain ACT Sin is only accurate to |arg| <~ pi, so EVERY sine needs this)
  - round(y) via the +-1.5*2^23 magic-number trick on DVE (no round ALU op);
    hidden layers use the single-op ADD_RANGE_WRAP (|y| <= 1.5 guaranteed,
    host-checked ~0.3), the encoding uses magic-round (|y| up to 256)
  - matmuls in float32r (1 cycle/col at free dim >= 256; ~2^-11 products)
  - x broadcast to 60 partitions with a step-0 DMA access pattern
  - W4[:,1:]@W5[:15] folded into one [128,128] matmul (no nonlinearity
    between them), so the 15 scalar features are never materialized
"""
import sys

sys.path.insert(0, "/opt/trn_rl_repo")

import numpy as np
import concourse.bass as bass
import concourse.bacc as bacc
import concourse.tile as tile
from concourse import mybir
from concourse.bass_utils import run_bass_kernel_spmd

F32 = mybir.dt.float32
F32R = mybir.dt.float32r
AF = mybir.ActivationFunctionType
ALU = mybir.AluOpType

N = 1048576
NCORES = 8
NPC = N // NCORES          # 131072 points per core
T = 512                    # points per tile (one PSUM bank at fp32)
TILES = NPC // T           # 256
H = 128
NUM_FREQ = 10
ENC_SIN = 6 * NUM_FREQ     # 60 sin/cos rows
OMEGA = 30.0
TWO_PI = 2.0 * np.pi
MAGIC = float(np.float32(1.5 * 2 ** 23))   # fp32 round-to-nearest-int trick
STAGE_TILES = 8            # tiles per output staging buffer

_compiled = None


def _build():
    nc = bacc.Bacc("TRN2", target_bir_lowering=False)

    xT = nc.dram_tensor("xT", [3, NPC], F32, kind="ExternalInput")
    wspec = {
        "w0x": [3, H], "w0s": [ENC_SIN, H],
        "w1": [H, H], "w2": [H, H], "w3": [H, H],
        "w4c": [H, 1],
        "w45": [H, H], "w5x": [3, H], "w5s": [ENC_SIN, H],
        "w6": [H, H], "w7": [H, 1],
    }
    wdram = {k: nc.dram_tensor(k, shp, F32R, kind="ExternalInput") for k, shp in wspec.items()}
    # per-partition constants for the encoding rows and layer biases
    encscale = nc.dram_tensor("encscale", [ENC_SIN, 1], F32, kind="ExternalInput")
    encshift = nc.dram_tensor("encshift", [ENC_SIN, 1], F32, kind="ExternalInput")
    encbias = nc.dram_tensor("encbias", [ENC_SIN, 1], F32, kind="ExternalInput")
    lbias = nc.dram_tensor("lbias", [H, 6], F32, kind="ExternalInput")  # 30*b per sine layer
    scal2 = nc.dram_tensor("scal2", [1, 4], F32, kind="ExternalInput")  # [b4_0, b7_0, _, _]
    s_out = nc.dram_tensor("s_out", [1, NPC], F32, kind="ExternalOutput")
    d_out = nc.dram_tensor("d_out", [1, NPC], F32, kind="ExternalOutput")

    with tile.TileContext(nc) as tc:
        with (
            tc.tile_pool(name="wpool", bufs=1) as wpool,
            tc.tile_pool(name="inp", bufs=3) as inp,
            tc.tile_pool(name="encp", bufs=2) as encp,
            tc.tile_pool(name="hid", bufs=4) as hid,
            tc.tile_pool(name="stage", bufs=2) as stagep,
            tc.tile_pool(name="yps", bufs=4, space="PSUM") as yps,
            tc.tile_pool(name="sps", bufs=2, space="PSUM") as sps,
        ):
            wt = {}
            for k, shp in wspec.items():
                wt[k] = wpool.tile(shp, F32R, name=f"wt_{k}")
                nc.sync.dma_start(out=wt[k], in_=wdram[k][:, :])
            esc = wpool.tile([ENC_SIN, 1], F32)
            nc.sync.dma_start(out=esc, in_=encscale[:, :])
            esh = wpool.tile([ENC_SIN, 1], F32)
            nc.sync.dma_start(out=esh, in_=encshift[:, :])
            ebi = wpool.tile([ENC_SIN, 1], F32)
            nc.sync.dma_start(out=ebi, in_=encbias[:, :])
            lbi = wpool.tile([H, 6], F32)
            nc.sync.dma_start(out=lbi, in_=lbias[:, :])
            sc2 = wpool.tile([1, 4], F32)
            nc.sync.dma_start(out=sc2, in_=scal2[:, :])

            stage_d = stage_s = None
            for it in range(TILES):
                col = it * T
                if it % STAGE_TILES == 0:
                    stage_d = stagep.tile([1, STAGE_TILES * T], F32, name="stage_d")
                    stage_s = stagep.tile([1, STAGE_TILES * T], F32, name="stage_s")
                scol = (it % STAGE_TILES) * T

                # ---- encoding: y = 2^(k-1) * x (exact), d = y - round(y+shift)
                x_bc = inp.tile([ENC_SIN, T], F32)
                nc.sync.dma_start(
                    out=x_bc,
                    in_=bass.AP(tensor=xT, offset=col, ap=[[0, 20], [NPC, 3], [1, T]]),
                )
                xr = inp.tile([3, T], F32R)
                nc.vector.tensor_copy(xr, x_bc[0:3, :])

                y_e = encp.tile([ENC_SIN, T], F32)
                nc.vector.tensor_scalar(out=y_e, in0=x_bc, scalar1=esc[:, 0:1],
                                        scalar2=None, op0=ALU.mult)
                t_e = encp.tile([ENC_SIN, T], F32)
                nc.vector.tensor_scalar(out=t_e, in0=y_e, scalar1=esh[:, 0:1],
                                        scalar2=MAGIC, op0=ALU.add, op1=ALU.add)
                k_e = encp.tile([ENC_SIN, T], F32)
                nc.vector.tensor_scalar(out=k_e, in0=t_e, scalar1=MAGIC,
                                        scalar2=None, op0=ALU.subtract)
                d_e = encp.tile([ENC_SIN, T], F32)
                nc.vector.tensor_tensor(out=d_e, in0=y_e, in1=k_e, op=ALU.subtract)
                encs = encp.tile([ENC_SIN, T], F32R)
                nc.scalar.activation(encs, d_e, AF.Sin, bias=ebi[:, 0:1], scale=TWO_PI)

                def sine_layer(yp, li):
                    wr_ = hid.tile([H, T], F32)
                    nc.vector.add_range_wrap(wr_, yp, 0.0, 0.5, 1.0)
                    h_ = hid.tile([H, T], F32R)
                    nc.scalar.activation(h_, wr_, AF.Sin, bias=lbi[:, li:li + 1], scale=TWO_PI)
                    return h_

                # ---- L0
                yp = yps.tile([H, T], F32)
                nc.tensor.matmul(yp, wt["w0x"], xr, start=True, stop=False)
                nc.tensor.matmul(yp, wt["w0s"], encs, start=False, stop=True)
                h = sine_layer(yp, 0)
                # ---- L1..L3
                for li, wk in ((1, "w1"), (2, "w2"), (3, "w3")):
                    yp = yps.tile([H, T], F32)
                    nc.tensor.matmul(yp, wt[wk], h, start=True, stop=True)
                    h = sine_layer(yp, li)
                h3 = h
                # ---- density row: relu(W4[:,0]^T h3 + b4_0)
                dp = sps.tile([1, T], F32)
                nc.tensor.matmul(dp, wt["w4c"], h3, start=True, stop=True)
                nc.vector.tensor_scalar(out=stage_d[0:1, scol:scol + T], in0=dp,
                                        scalar1=sc2[0:1, 0:1], scalar2=0.0,
                                        op0=ALU.add, op1=ALU.max)
                # ---- L5 = sin(30*([scal, enc] @ W5 + b5')): W45 folds W4[:,1:]@W5[:15]
                yp = yps.tile([H, T], F32)
                nc.tensor.matmul(yp, wt["w45"], h3, start=True, stop=False)
                nc.tensor.matmul(yp, wt["w5x"], xr, start=False, stop=False)
                nc.tensor.matmul(yp, wt["w5s"], encs, start=False, stop=True)
                h = sine_layer(yp, 4)
                # ---- L6
                yp = yps.tile([H, T], F32)
                nc.tensor.matmul(yp, wt["w6"], h, start=True, stop=True)
                h6 = sine_layer(yp, 5)
                # ---- L7: s = W7^T h6 + b7
                sp = sps.tile([1, T], F32)
                nc.tensor.matmul(sp, wt["w7"], h6, start=True, stop=True)
                nc.vector.tensor_scalar(out=stage_s[0:1, scol:scol + T], in0=sp,
                                        scalar1=sc2[0:1, 1:2], scalar2=None, op0=ALU.add)

                if (it + 1) % STAGE_TILES == 0:
                    base = (it + 1 - STAGE_TILES) * T
                    w = STAGE_TILES * T
                    nc.sync.dma_start(out=d_out[0:1, base:base + w], in_=stage_d[0:1, :])
                    nc.sync.dma_start(out=s_out[0:1, base:base + w], in_=stage_s[0:1, :])

    nc.compile()
    return nc


def _prep_host(inputs):
    """Host-side weight prep in float64, returns the per-core input maps."""
    f8 = {k: np.asarray(v, dtype=np.float64) for k, v in inputs.items()}
    W0, W1, W2, W3, W4, W5, W6, W7 = (f8[f"W{i}"] for i in range(8))
    b0, b1, b2, b3, b4, b5, b6, b7 = (f8[f"b{i}"] for i in range(8))
    SC = OMEGA / TWO_PI

    w = {}
    w["w0x"] = W0[0:3] * SC
    w["w0s"] = W0[3:63] * SC
    w["w1"] = W1 * SC
    w["w2"] = W2 * SC
    w["w3"] = W3 * SC
    w["w4c"] = W4[:, 0:1]
    w["w45"] = (W4[:, 1:16] @ W5[0:15]) * SC
    w["w5x"] = W5[15:18] * SC
    w["w5s"] = W5[18:78] * SC
    w["w6"] = W6 * SC
    w["w7"] = W7
    w = {k: np.ascontiguousarray(v, dtype=np.float32) for k, v in w.items()}

    # encoding row constants: row r = k*6+j; j<3 -> sin(2^k pi x_j), else cos
    escale = np.zeros((ENC_SIN, 1), np.float32)
    eshift = np.zeros((ENC_SIN, 1), np.float32)
    ebias = np.zeros((ENC_SIN, 1), np.float32)
    for k in range(NUM_FREQ):
        for j in range(6):
            r = k * 6 + j
            escale[r] = 2.0 ** (k - 1)      # y = 2^(k-1) x, so 2*pi*y = 2^k pi x
            if j >= 3:                       # cos row: sin(z + pi/2)
                eshift[r] = 0.25             # round(y + 1/4) keeps arg in [-pi, pi]
                ebias[r] = np.pi / 2

    # effective sine-layer biases (in radians, added post-wrap inside ACT Sin)
    b5p = b5 + b4[1:16] @ W5[0:15]
    lb = np.zeros((H, 6), np.float32)
    for i, b in enumerate([b0, b1, b2, b3, b5p, b6]):
        lb[:, i] = (OMEGA * b).astype(np.float32)
    sc2 = np.array([[b4[0], b7[0], 0.0, 0.0]], np.float32)

    xT_full = np.ascontiguousarray(np.asarray(inputs["input_points"], np.float32).T)  # [3, N]
    in_maps = []
    for c in range(NCORES):
        m = {k: v for k, v in w.items()}
        m["encscale"] = escale
        m["encshift"] = eshift
        m["encbias"] = ebias
        m["lbias"] = lb
        m["scal2"] = sc2
        m["xT"] = np.ascontiguousarray(xT_full[:, c * NPC:(c + 1) * NPC])
        in_maps.append(m)
    return in_maps


def kernel(**inputs):
    global _compiled
    if _compiled is None:
        _compiled = _build()
    nc = _compiled
    in_maps = _prep_host(inputs)
    res = run_bass_kernel_spmd(nc, in_maps, list(range(NCORES)))
    s = np.concatenate([r["s_out"].reshape(-1) for r in res.results]).reshape(N, 1)
    d = np.concatenate([r["d_out"].reshape(-1) for r in res.results]).reshape(N)
    return s.astype(np.float32), d.astype(np.float32)
